# revision 46
# baseline (speedup 1.0000x reference)
"""Trainium2 Bass kernel for nn_Attention_27719718929033.

Channel-attention block + 3x3 conv, data-parallel over batch across 8 cores.

Attention (per batch, X = x[b] in [C, N], N = H*W = 4096):
    logits = Wq_s (X X^T) Wk^T  (Gram identity; X^T tiles via PE transposes)
    A = softmax_rows(logits); out2 = X^T (proj_w A Wv)^T  token-major [N, C]
    reference reshapes [N,C]->[C,H,W] by flat reinterpretation, done here via
    a DRAM round-trip re-read in [C, HW] layout.

Conv via 1D-horizontal Winograd F(2,3) (1.5x fewer PE columns than direct):
    V planes (4 per chunk) from column combos of x; M_v accumulates over
    (dy, c_in) in PSUM via row-shifted APs; epilogue fuses the inverse
    transform with the attention add: y_even = ar + M0+M1+M2,
    y_odd = ar + M1-M2-M3. Weight transform (G w) is precomputed on host.

Layout: x is cast to bf16 and stored DE-INTERLEAVED by column parity at DMA
assembly time (ACT engine). Gram/logits are token-permutation-invariant, so
attention consumes the even/odd tiles directly; out2 compensates with a
strided DRAM write; all Winograd V ops then read contiguously (DVE strided
bf16 is 4x slower than contiguous). Matmuls run bf16 / fp32r (both 1 col per
cycle at free >= 256) with fp32 PSUM accumulation; rel err ~3.5e-3 vs the
fp32 reference (gate 2e-2).

Schedule: phases of the two batches are zippered so every serial PSUM->SBUF
copy latency is covered by independent PE work; Winograd groups double as
fillers behind softmax/U/MT chains; PSUM banks are shared between attention
and Winograd pools by rotating pair tiles through both.
"""
from contextlib import ExitStack

import ml_dtypes
import numpy as np

import concourse.bacc as bacc
import concourse.mybir as mybir
import concourse.tile as tile
from concourse.bass_utils import run_bass_kernel_spmd
from concourse.tile_rust import add_dep_helper

N_CORES = 8
B, C, H, W = 16, 256, 64, 64
BL = B // N_CORES  # batches per core
N = H * W  # tokens
HP = H + 2  # padded
WP = W + 2
CK = C // 128  # channel chunks of 128
TT = N // 128  # token tiles of 128
XS = 8  # x_sb sub-tiles per (batch, chunk) so compute starts early
HT = H // 8  # h-tiles of 8 rows (free dim 8*64 = 512)
SCALE = C ** (-0.5)

F32 = mybir.dt.float32
F32R = mybir.dt.float32r
BF16 = mybir.dt.bfloat16


def build_program(use_qkv_bias, use_v_bias, use_proj_bias, use_conv_bias):
    nc = bacc.Bacc(None, target_bir_lowering=False)

    x = nc.declare_dram_parameter("x", [BL, C, N], F32, isOutput=False)
    wqk_t = nc.declare_dram_parameter("wqk_t", [C, 2 * C], F32R, isOutput=False)
    wv = nc.declare_dram_parameter("wv", [C, C], F32R, isOutput=False)
    pw_t = nc.declare_dram_parameter("pw_t", [C, C], F32R, isOutput=False)
    uw = nc.declare_dram_parameter("uw", [12, C, C], mybir.dt.bfloat16, isOutput=False)
    bqk = bv = pb = cb = None
    if use_qkv_bias:
        bqk = nc.declare_dram_parameter("bqk", [2 * C], F32R, isOutput=False)
    if use_v_bias:
        bv = nc.declare_dram_parameter("bv", [C], F32R, isOutput=False)
    if use_proj_bias:
        pb = nc.declare_dram_parameter("pb", [C], F32R, isOutput=False)
    if use_conv_bias:
        cb = nc.declare_dram_parameter("cb", [C], F32, isOutput=False)
    ident = nc.declare_dram_parameter("ident", [128, 128], mybir.dt.bfloat16, isOutput=False)
    out = nc.declare_dram_parameter("out", [BL, C, N], F32, isOutput=True)

    attn_dram = nc.dram_tensor("attn_scratch", [BL, N, C], mybir.dt.bfloat16)

    with tile.TileContext(nc) as tc, ExitStack() as ctx:
        # --- persistent SBUF pools ---
        xp_pool = ctx.enter_context(tc.tile_pool(name="ximg", bufs=1))
        stg_pool = ctx.enter_context(tc.tile_pool(name="xstage", bufs=4))
        w_pool = ctx.enter_context(tc.tile_pool(name="weights", bufs=1))
        qk_pool = ctx.enter_context(tc.tile_pool(name="qk", bufs=6))
        sm_pool = ctx.enter_context(tc.tile_pool(name="smx", bufs=2))
        attn_pool = ctx.enter_context(tc.tile_pool(name="attnmat", bufs=2))
        o2_pool = ctx.enter_context(tc.tile_pool(name="o2", bufs=4))
        ard_pool = ctx.enter_context(tc.tile_pool(name="attnrd", bufs=4))
        cvo_pool = ctx.enter_context(tc.tile_pool(name="convout", bufs=4))
        # PSUM pools: (3 + 3 + 2) banks of 8
        cv_ps_pool = ctx.enter_context(
            tc.tile_pool(name="cvps", bufs=2, space="PSUM")
        )
        mm_ps_pool = ctx.enter_context(
            tc.tile_pool(name="mmps", bufs=4, space="PSUM")
        )
        at_ps_pool = ctx.enter_context(
            tc.tile_pool(name="atps", bufs=2, space="PSUM")
        )

        # --- weights to SBUF ---
        wqk_sb = w_pool.tile([128, CK, 2 * C], F32R, tag="wqk")
        wv_sb = w_pool.tile([128, CK, C], F32R, tag="wv")
        pw_sb = w_pool.tile([128, CK, C], F32R, tag="pw")
        uw_sb = [
            w_pool.tile([128, 12, C], BF16, tag=f"uw{ic}", name=f"uw_sb{ic}")
            for ic in range(CK)
        ]
        ident_sb = w_pool.tile([128, 128], BF16, tag="ident")
        nc.sync.dma_start(ident_sb[:], ident[:])
        # pre-warm the Exp activation table during DMA lead-in so softmax
        # doesn't pay the 1.3us ACT_TABLE_LOAD on the critical chain
        warm = w_pool.tile([1, 2], F32, tag="warm")
        nc.vector.memset(warm[:], 0.0)
        nc.scalar.activation(warm[:], warm[:], mybir.ActivationFunctionType.Exp)

        ones1 = None
        if use_qkv_bias or use_v_bias or use_proj_bias:
            ones1 = w_pool.tile([1, 128], F32R, tag="ones")
            nc.gpsimd.memset(ones1[:].bitcast(F32), 1.0)
        bqk_sb = None
        if use_qkv_bias:
            bqk_sb = w_pool.tile([1, 2 * C], F32R, tag="bqk")
            nc.sync.dma_start(bqk_sb[:], bqk[:].rearrange("c -> 1 c"))
        bv_sb = None
        if use_v_bias:
            bv_sb = w_pool.tile([128, CK], F32R, tag="bv")
            for dc in range(CK):
                nc.sync.dma_start(
                    bv_sb[:, dc], bv[dc * 128 : (dc + 1) * 128].rearrange("p -> p 1")
                )
        pb_sb = None
        if use_proj_bias:
            pb_sb = w_pool.tile([1, C], F32R, tag="pb")
            nc.sync.dma_start(pb_sb[:], pb[:].rearrange("c -> 1 c"))
        cb_sb = None
        if use_conv_bias:
            cb_sb = w_pool.tile([128, CK], F32, tag="cb")
            for oc in range(CK):
                nc.sync.dma_start(
                    cb_sb[:, oc], cb[oc * 128 : (oc + 1) * 128].rearrange("p -> p 1")
                )

        # --- input: DMA lands fp32 sub-tiles in a small staging pool;
        # GpSimd casts/assembles them into one contiguous bf16 image tile
        # x_full[b][ck] = [128, N]. Attention (transposes/out2 stationaries)
        # and the Winograd-1D input transform both read x_full, so no padded
        # image is kept at all (horizontal edges are AP-handled, vertical
        # edges become partial-range matmuls). ---
        NS = N // XS  # tokens per DMA sub-tile
        # x is stored de-interleaved by column parity: x_eo[.., 0, :] holds
        # even image columns, x_eo[.., 1, :] odd ones (bf16). The gram/logits
        # accumulation is token-permutation-invariant, so attention consumes
        # E/O tiles directly; only out2's DRAM writes need a strided target.
        # The payoff: every Winograd V-transform op reads contiguously (DVE
        # strided bf16 reads measured 4x slower than contiguous).
        x_eo = [
            [
                xp_pool.tile(
                    [128, 2, N // 2], BF16, tag=f"x{b}{ck}", name=f"x_eo{b}_{ck}"
                )
                for ck in range(CK)
            ]
            for b in range(BL)
        ]
        # All x assembly (fp32 stage -> bf16 x_full) runs on ACT, which is
        # otherwise idle during the gram phases. Batch 0 is emitted up front;
        # batch 1 is drip-fed through g_gram(0)'s loop so the copies never
        # sit in front of attention-critical ACT work.
        # 1024-token stage tiles halve the Sync descriptor-gen load of the
        # x stream. Even columns (consumed first by the gram E-windows) are
        # extracted on ACT; odd columns have a later deadline and go to the
        # otherwise-idle GpSimd.
        SS = 4  # stage sub-tiles per (batch, chunk)
        NSS = N // SS
        def emit_asm(b, s, ck):
            stg = stg_pool.tile([128, NSS], F32, tag="stg")
            nc.sync.dma_start(
                stg[:],
                x[b, ck * 128 : (ck + 1) * 128, s * NSS : (s + 1) * NSS],
            )
            h = NSS // 2
            nc.scalar.copy(
                x_eo[b][ck][:, 0, s * h : (s + 1) * h], stg[:, 0:NSS:2]
            )
            if b == 0:
                nc.vector.tensor_copy(
                    x_eo[b][ck][:, 1, s * h : (s + 1) * h], stg[:, 1:NSS:2]
                )
            else:
                nc.scalar.copy(
                    x_eo[b][ck][:, 1, s * h : (s + 1) * h], stg[:, 1:NSS:2]
                )

        for s in range(SS):
            for ck in range(CK):
                emit_asm(0, s, ck)
        b1_asm = [
            (lambda s=s, ck=ck: emit_asm(1, s, ck))
            for s in range(SS)
            for ck in range(CK)
        ]
        for kc in range(CK):
            nc.sync.dma_start(
                wqk_sb[:, kc, :], wqk_t[kc * 128 : (kc + 1) * 128, :]
            )
            nc.sync.dma_start(
                wv_sb[:, kc, :], wv[kc * 128 : (kc + 1) * 128, :]
            )
            nc.sync.dma_start(
                pw_sb[:, kc, :], pw_t[kc * 128 : (kc + 1) * 128, :]
            )
        for kc in range(CK):
            nc.sync.dma_start(
                uw_sb[kc][:],
                uw[:, kc * 128 : (kc + 1) * 128, :].rearrange("t p o -> p t o"),
            )

        def tok_window(b, ck, t):
            # lhsT (stationary): [128 chan, 128 tokens] bf16. t in [0,16) maps
            # to even-column tokens, [16,32) to odd ones (a fixed permutation
            # of the token axis -- transparent to gram/logits, and out2
            # compensates with a strided DRAM write).
            par, tt = divmod(t, TT // 2)
            return x_eo[b][ck][:, par, tt * 128 : (tt + 1) * 128]

        mt_sbs = {}
        r_sbs = {}
        lg_pss = {}
        a_sbs = {}

        wqkb_holder = {}

        def qk_phase(b):
            # ---- fused [Q|K] + logits ----
            if "t" not in wqkb_holder:
                wqkb = w_pool.tile([128, CK, 2 * C], BF16, tag="wqkb")
                for kc in range(CK):
                    nc.vector.tensor_copy(wqkb[:, kc, :], wqk_sb[:, kc, :].bitcast(F32))
                wqkb_holder["t"] = wqkb
            wqkb = wqkb_holder["t"]
            lg_ps = at_ps_pool.tile(
                [128, CK, C], F32, tag="atps", name=f"lg_ps{b}"
            )
            for t in range(TT):
                qk_ps = mm_ps_pool.tile([128, 2 * C], F32, tag="qkps")
                for kc in range(CK):
                    nc.tensor.matmul(
                        qk_ps[:],
                        tok_window(b, kc, t),
                        wqkb[:, kc, :],
                        start=(kc == 0),
                        stop=(kc == CK - 1 and not use_qkv_bias),
                    )
                if use_qkv_bias:
                    nc.tensor.matmul(
                        qk_ps[:], ones1[:], bqk_sb[:], start=False, stop=True
                    )
                qk_sb = qk_pool.tile([128, 2 * C], F32R, tag="qksb")
                nc.vector.tensor_copy(qk_sb[:], qk_ps[:].bitcast(F32R))

                for cc in range(CK):
                    mm = nc.tensor.matmul(
                        lg_ps[:, cc, :],
                        qk_sb[:, cc * 128 : (cc + 1) * 128],
                        qk_sb[:, C : 2 * C],
                        start=(t == 0 and cc == 0),
                        stop=(t == TT - 1),
                        skip_group_check=True,
                    )
                    # start=True clears the WHOLE bank; order sibling groups
                    # after the clearing matmul
                    if t == 0 and cc == 0:
                        lg_clear = mm
                    elif t == 0:
                        add_dep_helper(
                            mm.ins, lg_clear.ins, sync=False,
                            reason="after lg bank clear",
                        )

            lg_pss[b] = lg_ps

        # --- Gram/logits phase, split into parts so the scheduler can
        # zipper batches: transposes are emitted ahead of gram matmuls, and
        # the serial g_sb/t1_sb copy latencies of one batch are covered by
        # independent PE work of the other. ---
        g_state = {}
        GDEPTH = 2  # transpose lookahead over gram consumption

        def g_init(b):
            # NOTE: must be called only after the previous batch's g_sb copy
            # (chain1) is emitted -- at_ps has bufs=2. Transposes may already
            # have been emitted for this batch; keep their state.
            st = g_state.setdefault(b, dict(xt={}, nextT=0))
            st["g_ps"] = at_ps_pool.tile(
                [128, CK, C], F32, tag="atps", name=f"g_ps{b}"
            )
            st["g_clear"] = None

        def g_transposes(b, upto):
            st = g_state.setdefault(b, dict(xt={}, nextT=0))
            while st["nextT"] < min(upto, TT):
                t = st["nextT"]
                st["nextT"] += 1
                xt_ps = mm_ps_pool.tile([128, C], BF16, tag="qkps")
                tclear = None
                for ck in range(CK):
                    mm = nc.tensor.matmul(
                        xt_ps[:, ck * 128 : (ck + 1) * 128],
                        tok_window(b, ck, t),
                        ident_sb[:],
                        is_transpose=True,
                        start=(ck == 0),
                        stop=(ck == CK - 1),
                        skip_group_check=True,
                    )
                    if ck == 0:
                        tclear = mm
                    else:
                        add_dep_helper(
                            mm.ins, tclear.ins, sync=False,
                            reason="after xt bank clear",
                        )
                xt_sb = qk_pool.tile([128, C], BF16, tag="qksb")
                nc.vector.tensor_copy(xt_sb[:], xt_ps[:])
                st["xt"][t] = xt_sb

        def g_gram(b, feed=None):
            st = g_state[b]
            for t in range(TT):
                if feed and t % 4 == 1:
                    feed.pop(0)()
                g_transposes(b, t + 1 + GDEPTH)
                xt_sb = st["xt"].pop(t)
                for cc in range(CK):
                    mm = nc.tensor.matmul(
                        st["g_ps"][:, cc, :],
                        xt_sb[:, cc * 128 : (cc + 1) * 128],
                        xt_sb[:],
                        start=(t == 0 and cc == 0),
                        stop=(t == TT - 1),
                        skip_group_check=True,
                    )
                    if t == 0 and cc == 0:
                        st["g_clear"] = mm
                    elif t == 0:
                        add_dep_helper(
                            mm.ins, st["g_clear"].ins, sync=False,
                            reason="after g bank clear",
                        )

        def g_chain1(b):
            # g_ps -> g_sb, then T1 = G Wk^T
            st = g_state[b]
            g_ps = st["g_ps"]
            g_sb = attn_pool.tile([128, CK, C], F32R, tag="g", name=f"g_sb{b}")
            nc.vector.tensor_copy(g_sb[:, 0, :], g_ps[:, 0, :].bitcast(F32R))
            nc.vector.tensor_copy(g_sb[:, 1, :], g_ps[:, 1, :].bitcast(F32R))
            t1_ps = mm_ps_pool.tile(
                [128, CK, C], F32, tag="qkps", name=f"t1_ps{b}"
            )
            t1_clear = None
            for cpc in range(CK):
                for dc in range(CK):
                    mm = nc.tensor.matmul(
                        t1_ps[:, cpc, :],
                        g_sb[:, dc, cpc * 128 : (cpc + 1) * 128],
                        wqk_sb[:, dc, C : 2 * C],
                        start=(cpc == 0 and dc == 0),
                        stop=(dc == CK - 1),
                        skip_group_check=True,
                    )
                    if cpc == 0 and dc == 0:
                        t1_clear = mm
                    elif dc == 0:
                        add_dep_helper(
                            mm.ins, t1_clear.ins, sync=False,
                            reason="after t1 bank clear",
                        )
            st["t1_ps"] = t1_ps

        def g_chain2(b):
            # t1_ps -> t1_sb, then logits = Wq_s T1
            st = g_state[b]
            t1_ps = st.pop("t1_ps")
            t1_sb = attn_pool.tile([128, CK, C], F32R, tag="t1", name=f"t1_sb{b}")
            nc.vector.tensor_copy(t1_sb[:, 0, :], t1_ps[:, 0, :].bitcast(F32R))
            nc.vector.tensor_copy(t1_sb[:, 1, :], t1_ps[:, 1, :].bitcast(F32R))
            lg_ps = at_ps_pool.tile(
                [128, CK, C], F32, tag="atps", name=f"glg_ps{b}"
            )
            lg_clear = None
            for cc in range(CK):
                for kc in range(CK):
                    mm = nc.tensor.matmul(
                        lg_ps[:, cc, :],
                        wqk_sb[:, kc, cc * 128 : (cc + 1) * 128],
                        t1_sb[:, kc, :],
                        start=(cc == 0 and kc == 0),
                        stop=(kc == CK - 1),
                        skip_group_check=True,
                    )
                    if cc == 0 and kc == 0:
                        lg_clear = mm
                    elif kc == 0:
                        add_dep_helper(
                            mm.ins, lg_clear.ins, sync=False,
                            reason="after glg bank clear",
                        )
            lg_pss[b] = lg_ps

        def g_phase(b):
            g_init(b)
            g_gram(b)
            g_chain1(b)
            g_chain2(b)

        def softmax_phase(b):
            lg_ps = lg_pss[b]
            # ---- softmax rows of logits -> A  [c part, d free] ----
            a_sb = attn_pool.tile([128, CK, C], F32R, tag="a", name=f"a_sb{b}")
            for cc in range(CK):
                ex = sm_pool.tile([128, C], F32, tag="ex")
                nmx = sm_pool.tile([128, 1], F32, tag=f"nmx{cc}", name=f"nmx{b}_{cc}")
                nc.vector.reduce_max(
                    nmx[:], lg_ps[:, cc, :], axis=mybir.AxisListType.X, negate=True
                )
                sm = sm_pool.tile([128, 1], F32, tag=f"sm{cc}", name=f"sm{b}_{cc}")
                nc.scalar.activation(
                    ex[:],
                    lg_ps[:, cc, :],
                    mybir.ActivationFunctionType.Exp,
                    bias=nmx[:],
                    scale=1.0,
                    accum_out=sm[:],
                )
                rs = sm_pool.tile([128, 1], F32, tag=f"rs{cc}", name=f"rs{b}_{cc}")
                nc.vector.reciprocal(rs[:], sm[:])
                nc.vector.tensor_scalar_mul(a_sb[:, cc, :], ex[:], rs[:])

            a_sbs[b] = a_sb

        u_sbs = {}

        def u_part(b):
            a_sb = a_sbs[b]
            # ---- U = A^T P^T : U[d, e] = sum_c A[c, d] pw_t[c, e] ----
            u_sb = attn_pool.tile([128, CK, C], F32R, tag="u", name=f"u_sb{b}")
            u_ps = mm_ps_pool.tile([128, CK, C], F32, tag="qkps", name=f"u_ps{b}")
            for dc in range(CK):
                for cc in range(CK):
                    mm = nc.tensor.matmul(
                        u_ps[:, dc, :],
                        a_sb[:, cc, dc * 128 : (dc + 1) * 128],
                        pw_sb[:, cc, :],
                        start=(dc == 0 and cc == 0),
                        stop=(cc == CK - 1),
                        skip_group_check=True,
                    )
                    if dc == 0 and cc == 0:
                        u_clear = mm
                    elif cc == 0:
                        add_dep_helper(
                            mm.ins, u_clear.ins, sync=False,
                            reason="after u bank clear",
                        )
            nc.vector.tensor_copy(u_sb[:, 0, :], u_ps[:, 0, :].bitcast(F32R))
            nc.scalar.copy(u_sb[:, 1, :], u_ps[:, 1, :].bitcast(F32R))
            u_sbs[b] = u_sb

        def mt_part(b):
            u_sb = u_sbs[b]
            # ---- M^T = Wv^T U : M^T[c', e] = sum_d wv[d, c'] U[d, e] ----
            mt_sb = attn_pool.tile([128, CK, C], BF16, tag="mt", name=f"mt_sb{b}")
            mt_ps = mm_ps_pool.tile(
                [128, CK, C], F32, tag="qkps", name=f"mt_ps{b}"
            )
            for cpc in range(CK):
                for dc in range(CK):
                    mm = nc.tensor.matmul(
                        mt_ps[:, cpc, :],
                        wv_sb[:, dc, cpc * 128 : (cpc + 1) * 128],
                        u_sb[:, dc, :],
                        start=(cpc == 0 and dc == 0),
                        stop=(dc == CK - 1),
                        skip_group_check=True,
                    )
                    if cpc == 0 and dc == 0:
                        mt_clear = mm
                    elif dc == 0:
                        add_dep_helper(
                            mm.ins, mt_clear.ins, sync=False,
                            reason="after mt bank clear",
                        )
            nc.vector.tensor_copy(mt_sb[:, 0, :], mt_ps[:, 0, :])
            nc.scalar.copy(mt_sb[:, 1, :], mt_ps[:, 1, :])
            mt_sbs[b] = mt_sb

        def out2_part(b, v_feed=None):
            u_sb = u_sbs[b]
            mt_sb = mt_sbs[b]
            # ---- r^T = bv^T U + pb ----
            use_r = use_v_bias or use_proj_bias
            r_sb = None
            if use_r:
                r_ps = mm_ps_pool.tile([1, C], F32, tag="qkps")
                started = False
                if use_v_bias:
                    for dc in range(CK):
                        nc.tensor.matmul(
                            r_ps[:],
                            bv_sb[:, dc],
                            u_sb[:, dc, :],
                            start=(dc == 0),
                            stop=(dc == CK - 1 and not use_proj_bias),
                        )
                    started = True
                if use_proj_bias:
                    nc.tensor.matmul(
                        r_ps[:],
                        ones1[0:1, 0:1],
                        pb_sb[:],
                        start=not started,
                        stop=True,
                    )
                r_sb = attn_pool.tile([1, C], F32R, tag="r", name=f"r_sb{b}")
                nc.vector.tensor_copy(r_sb[:], r_ps[:].bitcast(F32R))
            r_sbs[b] = r_sb

            # ---- out2[n, e] = sum_c' X[c', n] M^T[c', e] (+ 1 r^T) ----
            # PSUM drain alternates ACT/DVE so neither queue backs up and the
            # qkps pool recycles fast enough to never stall the next U chain.
            # Two token tiles share one DMA (halves Sync descriptor-gen load).
            o_sb = None
            for t in range(TT):
                o_ps = mm_ps_pool.tile([128, C], F32, tag="qkps")
                for kc in range(CK):
                    nc.tensor.matmul(
                        o_ps[:],
                        tok_window(b, kc, t),
                        mt_sb[:, kc, :],
                        start=(kc == 0),
                        stop=(kc == CK - 1 and not use_r),
                    )
                if use_r:
                    nc.tensor.matmul(
                        o_ps[:], ones1[:], r_sb[:], start=False, stop=True
                    )
                if t % 2 == 0:
                    o_sb = o2_pool.tile([128, 2, C], BF16, tag="o2sb")
                    nc.scalar.copy(o_sb[:, 0, :], o_ps[:])
                else:
                    nc.vector.tensor_copy(o_sb[:, 1, :], o_ps[:])
                    par, tt = divmod(t - 1, TT // 2)
                    nc.sync.dma_start(
                        attn_dram[b]
                        .rearrange("(j two) c -> j two c", two=2)[
                            tt * 128 : (tt + 2) * 128, par, :
                        ]
                        .rearrange("(a m) c -> m a c", a=2),
                        o_sb[:],
                    )
                if v_feed and t % 4 == 3:
                    v_feed.pop(0)()

        def rest_phase(b):
            u_part(b)
            mt_part(b)
            out2_part(b)

        # ---- Winograd-1D F(2,3) conv: horizontal transform only.
        # y[:, i, 2j]   = M0 + M1 + M2
        # y[:, i, 2j+1] = M1 - M2 - M3
        # M_v[k, i, j] = sum_{dy, c} U[v, dy][c, k] * V_v[c, i + dy - 1, j]
        # V planes are row-local (per-row strided column combos of x), the
        # vertical taps accumulate in PSUM via row-shifted APs of the same
        # V planes, and vertical image edges become partial-range matmuls.
        v_tiles = [
            [
                [
                    xp_pool.tile(
                        [128, H, 32], BF16, tag=f"v{b}{ck}{v}",
                        name=f"v{b}_{ck}_{v}",
                    )
                    for v in range(4)
                ]
                for ck in range(CK)
            ]
            for b in range(BL)
        ]

        def wino_v_ops(b, eng=None):
            # V planes from de-interleaved E/O -- every op reads contiguously.
            #   V0[j] = O[j-1] - O[j]   (V0[0] = -O[0])
            #   V1    = E + O
            #   V2    = O - E
            #   V3[j] = E[j] - E[j+1]   (V3[31] = E[31])
            if eng is None:
                eng = nc.vector
            ops = []
            for ck in range(CK):
                ev = x_eo[b][ck][:, 0, :].rearrange("p (h j) -> p h j", h=H)
                ov = x_eo[b][ck][:, 1, :].rearrange("p (h j) -> p h j", h=H)
                v0, v1, v2, v3 = (v_tiles[b][ck][v][:] for v in range(4))
                ops.append(lambda v1=v1, ev=ev, ov=ov: eng.tensor_add(
                    v1[:, :, :], ev[:, :, :], ov[:, :, :]))
                ops.append(lambda v2=v2, ev=ev, ov=ov: eng.tensor_sub(
                    v2[:, :, :], ov[:, :, :], ev[:, :, :]))

                def v0_emit(v0=v0, ov=ov):
                    eng.tensor_sub(
                        v0[:, :, 1:32], ov[:, :, 0:31], ov[:, :, 1:32]
                    )
                    eng.tensor_scalar_mul(v0[:, :, 0:1], ov[:, :, 0:1], -1.0)
                ops.append(v0_emit)

                def v3_emit(v3=v3, ev=ev):
                    eng.tensor_sub(
                        v3[:, :, 0:31], ev[:, :, 0:31], ev[:, :, 1:32]
                    )
                    eng.tensor_copy(v3[:, :, 31:32], ev[:, :, 31:32])
                ops.append(v3_emit)
            return ops

        wino_ct = {}

        def wino_groups(b, tiles, v_feed=None):
            # 16-row groups: each M_v plane fills one full PSUM bank
            # ([128, 512], 512-free matmuls stream ~12% faster than 256-free).
            # M0/M1 come from the qkps pool (freed early in the drain),
            # M2/M3 from the cv pool (freed last) -- with 6 banks total this
            # keeps two groups in flight.
            attn_chw = attn_dram[b].rearrange("(p q) c -> p q c", p=C)
            for oc, hh in tiles:
                gi = wino_ct.get(b, 0)
                wino_ct[b] = gi + 1
                psM = []
                for v in range(4):
                    if v < 2:
                        pool, tag = mm_ps_pool, "qkps"
                    elif b > 0 and gi % 2 == 1:
                        # attention's at_ps banks are idle once softmax(1) is
                        # done -- rotating M2/M3 through them doubles the
                        # group pipeline depth in the pure-wino phase
                        pool, tag = at_ps_pool, "atps"
                    else:
                        pool, tag = cv_ps_pool, "cvps"
                    psM.append(
                        pool.tile([128, 512], F32, tag=tag, name=f"w{b}{oc}{hh}{v}")
                    )
                # prefetch the attention readback while the matmuls stream
                ar = ard_pool.tile([128, 4, C], BF16, tag="ar")
                nc.sync.dma_start(
                    ar[:],
                    attn_chw[oc * 128 : (oc + 1) * 128, hh * 4 : hh * 4 + 4, :],
                )
                for v in range(4):
                    clear = None
                    for dy in (1, 0, 2):
                        r0 = hh * 16 + dy - 1
                        lo, hi = max(r0, 0), min(r0 + 16, H)
                        for ic in range(CK):
                            mm = nc.tensor.matmul(
                                psM[v][:, (lo - r0) * 32 : (hi - r0) * 32],
                                uw_sb[ic][
                                    :, v * 3 + dy, oc * 128 : (oc + 1) * 128
                                ],
                                v_tiles[b][ic][v][:, lo:hi, :],
                                start=(clear is None),
                                stop=(dy == 2 and ic == CK - 1),
                                skip_group_check=True,
                            )
                            if clear is None:
                                clear = mm
                            else:
                                add_dep_helper(
                                    mm.ins, clear.ins, sync=False,
                                    reason="after wino bank clear",
                                )
                co = cvo_pool.tile([128, 16 * W], F32, tag="co")
                cov = co[:].rearrange("p (r w) -> p r w", r=16)
                arv = ar[:].rearrange("p a (r w) -> p (a r) w", r=4)
                m = [
                    psM[v][:].rearrange("p (r j) -> p r j", r=16)
                    for v in range(4)
                ]
                co_e = cov[:, :, 0:64:2]
                co_o = cov[:, :, 1:64:2]
                nc.vector.tensor_add(co_e, arv[:, :, 0:64:2], m[0])
                nc.vector.tensor_add(co_e, co_e, m[1])
                nc.vector.tensor_add(co_e, co_e, m[2])
                nc.vector.tensor_add(co_o, arv[:, :, 1:64:2], m[1])
                nc.vector.tensor_sub(co_o, co_o, m[2])
                nc.vector.tensor_sub(co_o, co_o, m[3])
                if use_conv_bias:
                    nc.vector.tensor_scalar_add(co[:], co[:], cb_sb[:, oc])
                nc.sync.dma_start(
                    out[
                        b,
                        oc * 128 : (oc + 1) * 128,
                        hh * 1024 : (hh + 1) * 1024,
                    ],
                    co[:],
                )
                if v_feed:
                    v_feed.pop(0)()

        def conv_phase(b, tiles=None, gp_add=False):
            all_tiles = [(oc, hh) for oc in range(CK) for hh in range(4)]
            wino_groups(b, tiles if tiles is not None else all_tiles)

        front = qk_phase if use_qkv_bias else g_phase
        if use_qkv_bias or BL == 1:
            for op in b1_asm:
                op()
            b1_asm = []
            front(0)
            softmax_phase(0)
            if BL > 1:
                front(1)
            rest_phase(0)
            if BL > 1:
                softmax_phase(1)
                rest_phase(1)
            for b in range(BL):
                for op in wino_v_ops(b):
                    op()
                conv_phase(b)
        else:
            # Optimized schedule. The PE queue is in-order, so every serial
            # copy latency (g_sb/t1_sb/u/mt PSUM->SBUF, softmax) is covered
            # by emitting independent PE work from the other batch or from
            # wino-conv(0) right behind it. conv(0) groups may only be
            # emitted after out2(0) (their attn readback depends on those
            # writes). V-transform DVE ops are drip-fed into DVE-slack spots.
            all_tiles0 = [(oc, hh) for oc in range(CK) for hh in range(4)]
            v0_ops = wino_v_ops(0)
            g_init(0)
            g_gram(0, feed=b1_asm)
            g_transposes(1, 3)   # covers g_sb(0) copy latency
            g_chain1(0)
            g_transposes(1, 6)   # covers t1_sb(0) copy latency
            g_chain2(0)
            softmax_phase(0)
            g_init(1)            # after chain1(0): at_ps has bufs=2
            g_gram(1)
            u_part(0)
            g_chain1(1)
            mt_part(0)
            g_chain2(1)
            out2_part(0, v_feed=v0_ops)
            softmax_phase(1)
            wino_groups(0, all_tiles0[0:1])
            u_part(1)
            wino_groups(0, all_tiles0[1:2])
            mt_part(1)
            wino_groups(0, all_tiles0[2:3])
            out2_part(1)
            wino_groups(0, all_tiles0[3:])
            # V(1) after the b0 groups: keeps DVE free for their PSUM drains
            # (V(1) is only needed by the b1 groups, ~30us later)
            for op in wino_v_ops(1):
                op()
            wino_groups(1, all_tiles0)

    nc.compile()
    return nc


def _prep_inputs(x, qkv_w, qkv_b, proj_w, proj_b, conv_w, conv_b):
    f = np.float32
    x = np.ascontiguousarray(x, dtype=f).reshape(B, C, N)
    qkv_w = np.asarray(qkv_w, dtype=f)
    qkv_b = np.asarray(qkv_b, dtype=f)
    proj_w = np.asarray(proj_w, dtype=f)
    proj_b = np.asarray(proj_b, dtype=f)
    conv_w = np.asarray(conv_w, dtype=f)
    conv_b = np.asarray(conv_b, dtype=f)

    # [Wq*s | Wk] transposed: [256 in, 512 out] (scale folded into Q side)
    wqk_t = np.ascontiguousarray(
        np.concatenate([(qkv_w[:C] * SCALE).T, qkv_w[C : 2 * C].T], axis=1)
    )
    wv = np.ascontiguousarray(qkv_w[2 * C :])
    pw_t = np.ascontiguousarray(proj_w.T)
    # Winograd-1D F(2,3) weight transform (horizontal): G @ w_taps
    g1 = np.array(
        [[1, 0, 0], [0.5, 0.5, 0.5], [0.5, -0.5, 0.5], [0, 0, 1]], dtype=f
    )
    uw = np.einsum("vx,kcyx->vyck", g1, conv_w).reshape(12, C, C)
    uw = np.ascontiguousarray(uw).astype(ml_dtypes.bfloat16)

    bqk = np.ascontiguousarray(
        np.concatenate([qkv_b[:C] * SCALE, qkv_b[C : 2 * C]])
    )
    bv = np.ascontiguousarray(qkv_b[2 * C :])

    flags = dict(
        use_qkv_bias=bool(np.any(bqk)),
        use_v_bias=bool(np.any(bv)),
        use_proj_bias=bool(np.any(proj_b)),
        use_conv_bias=bool(np.any(conv_b)),
    )
    shared = {
        "wqk_t": wqk_t,
        "wv": wv,
        "pw_t": pw_t,
        "uw": uw,
        "ident": np.eye(128, dtype=f).astype(ml_dtypes.bfloat16),
    }
    if flags["use_qkv_bias"]:
        shared["bqk"] = bqk
    if flags["use_v_bias"]:
        shared["bv"] = bv
    if flags["use_proj_bias"]:
        shared["pb"] = proj_b
    if flags["use_conv_bias"]:
        shared["cb"] = conv_b

    in_maps = []
    for core in range(N_CORES):
        m = dict(shared)
        m["x"] = np.ascontiguousarray(x[core * BL : (core + 1) * BL])
        in_maps.append(m)
    return in_maps, flags


def run(inputs, trace=False):
    in_maps, flags = _prep_inputs(**inputs)
    nc = build_program(**flags)
    res = run_bass_kernel_spmd(nc, in_maps, list(range(N_CORES)), trace=trace)
    out = np.concatenate(
        [res.results[i]["out"].reshape(BL, C, H, W) for i in range(N_CORES)], axis=0
    )
    return out, res


def kernel(**inputs):
    out, _ = run(inputs, trace=False)
    return out



# revision 47
# speedup vs baseline: 1.0143x; 1.0143x over previous
"""Trainium2 Bass kernel for nn_Attention_27719718929033.

Channel-attention block + 3x3 conv, data-parallel over batch across 8 cores.

Attention (per batch, X = x[b] in [C, N], N = H*W = 4096):
    logits = Wq_s (X X^T) Wk^T  (Gram identity; X^T tiles via PE transposes)
    A = softmax_rows(logits); out2 = X^T (proj_w A Wv)^T  token-major [N, C]
    reference reshapes [N,C]->[C,H,W] by flat reinterpretation, done here via
    a DRAM round-trip re-read in [C, HW] layout.

Conv via 1D-horizontal Winograd F(2,3) (1.5x fewer PE columns than direct):
    V planes (4 per chunk) from column combos of x; M_v accumulates over
    (dy, c_in) in PSUM via row-shifted APs; epilogue fuses the inverse
    transform with the attention add: y_even = ar + M0+M1+M2,
    y_odd = ar + M1-M2-M3. Weight transform (G w) is precomputed on host.

Layout: x is cast to bf16 and stored DE-INTERLEAVED by column parity at DMA
assembly time (ACT engine). Gram/logits are token-permutation-invariant, so
attention consumes the even/odd tiles directly; out2 compensates with a
strided DRAM write; all Winograd V ops then read contiguously (DVE strided
bf16 is 4x slower than contiguous). Matmuls run bf16 / fp32r (both 1 col per
cycle at free >= 256) with fp32 PSUM accumulation; rel err ~3.5e-3 vs the
fp32 reference (gate 2e-2).

Schedule: phases of the two batches are zippered so every serial PSUM->SBUF
copy latency is covered by independent PE work; Winograd groups double as
fillers behind softmax/U/MT chains; PSUM banks are shared between attention
and Winograd pools by rotating pair tiles through both.
"""
from contextlib import ExitStack

import ml_dtypes
import numpy as np

import concourse.bacc as bacc
import concourse.mybir as mybir
import concourse.tile as tile
from concourse.bass_utils import run_bass_kernel_spmd
from concourse.tile_rust import add_dep_helper

N_CORES = 8
B, C, H, W = 16, 256, 64, 64
BL = B // N_CORES  # batches per core
N = H * W  # tokens
HP = H + 2  # padded
WP = W + 2
CK = C // 128  # channel chunks of 128
TT = N // 128  # token tiles of 128
XS = 8  # x_sb sub-tiles per (batch, chunk) so compute starts early
HT = H // 8  # h-tiles of 8 rows (free dim 8*64 = 512)
SCALE = C ** (-0.5)

F32 = mybir.dt.float32
F32R = mybir.dt.float32r
BF16 = mybir.dt.bfloat16


def build_program(use_qkv_bias, use_v_bias, use_proj_bias, use_conv_bias):
    nc = bacc.Bacc(None, target_bir_lowering=False)

    x = nc.declare_dram_parameter("x", [BL, C, N], F32, isOutput=False)
    wqk_t = nc.declare_dram_parameter("wqk_t", [C, 2 * C], F32R, isOutput=False)
    wv = nc.declare_dram_parameter("wv", [C, C], F32R, isOutput=False)
    pw_t = nc.declare_dram_parameter("pw_t", [C, C], F32R, isOutput=False)
    uw = nc.declare_dram_parameter("uw", [12, C, C], mybir.dt.bfloat16, isOutput=False)
    bqk = bv = pb = cb = None
    if use_qkv_bias:
        bqk = nc.declare_dram_parameter("bqk", [2 * C], F32R, isOutput=False)
    if use_v_bias:
        bv = nc.declare_dram_parameter("bv", [C], F32R, isOutput=False)
    if use_proj_bias:
        pb = nc.declare_dram_parameter("pb", [C], F32R, isOutput=False)
    if use_conv_bias:
        cb = nc.declare_dram_parameter("cb", [C], F32, isOutput=False)
    ident = nc.declare_dram_parameter("ident", [128, 128], mybir.dt.bfloat16, isOutput=False)
    out = nc.declare_dram_parameter("out", [BL, C, N], F32, isOutput=True)

    attn_dram = nc.dram_tensor("attn_scratch", [BL, N, C], mybir.dt.bfloat16)

    with tile.TileContext(nc) as tc, ExitStack() as ctx:
        # --- persistent SBUF pools ---
        xp_pool = ctx.enter_context(tc.tile_pool(name="ximg", bufs=1))
        stg_pool = ctx.enter_context(tc.tile_pool(name="xstage", bufs=4))
        w_pool = ctx.enter_context(tc.tile_pool(name="weights", bufs=1))
        qk_pool = ctx.enter_context(tc.tile_pool(name="qk", bufs=6))
        sm_pool = ctx.enter_context(tc.tile_pool(name="smx", bufs=2))
        attn_pool = ctx.enter_context(tc.tile_pool(name="attnmat", bufs=2))
        o2_pool = ctx.enter_context(tc.tile_pool(name="o2", bufs=4))
        ard_pool = ctx.enter_context(tc.tile_pool(name="attnrd", bufs=4))
        cvo_pool = ctx.enter_context(tc.tile_pool(name="convout", bufs=4))
        # PSUM pools: (3 + 3 + 2) banks of 8
        cv_ps_pool = ctx.enter_context(
            tc.tile_pool(name="cvps", bufs=2, space="PSUM")
        )
        mm_ps_pool = ctx.enter_context(
            tc.tile_pool(name="mmps", bufs=4, space="PSUM")
        )
        at_ps_pool = ctx.enter_context(
            tc.tile_pool(name="atps", bufs=2, space="PSUM")
        )

        # --- weights to SBUF ---
        wqk_sb = w_pool.tile([128, CK, 2 * C], F32R, tag="wqk")
        wv_sb = w_pool.tile([128, CK, C], F32R, tag="wv")
        pw_sb = w_pool.tile([128, CK, C], F32R, tag="pw")
        uw_sb = [
            w_pool.tile([128, 12, C], BF16, tag=f"uw{ic}", name=f"uw_sb{ic}")
            for ic in range(CK)
        ]
        ident_sb = w_pool.tile([128, 128], BF16, tag="ident")
        # pre-warm the Exp activation table during DMA lead-in so softmax
        # doesn't pay the 1.3us ACT_TABLE_LOAD on the critical chain
        warm = w_pool.tile([1, 2], F32, tag="warm")
        nc.vector.memset(warm[:], 0.0)
        nc.scalar.activation(warm[:], warm[:], mybir.ActivationFunctionType.Exp)

        ones1 = None
        if use_qkv_bias or use_v_bias or use_proj_bias:
            ones1 = w_pool.tile([1, 128], F32R, tag="ones")
            nc.gpsimd.memset(ones1[:].bitcast(F32), 1.0)
        bqk_sb = None
        if use_qkv_bias:
            bqk_sb = w_pool.tile([1, 2 * C], F32R, tag="bqk")
            nc.sync.dma_start(bqk_sb[:], bqk[:].rearrange("c -> 1 c"))
        bv_sb = None
        if use_v_bias:
            bv_sb = w_pool.tile([128, CK], F32R, tag="bv")
            for dc in range(CK):
                nc.sync.dma_start(
                    bv_sb[:, dc], bv[dc * 128 : (dc + 1) * 128].rearrange("p -> p 1")
                )
        pb_sb = None
        if use_proj_bias:
            pb_sb = w_pool.tile([1, C], F32R, tag="pb")
            nc.sync.dma_start(pb_sb[:], pb[:].rearrange("c -> 1 c"))
        cb_sb = None
        if use_conv_bias:
            cb_sb = w_pool.tile([128, CK], F32, tag="cb")
            for oc in range(CK):
                nc.sync.dma_start(
                    cb_sb[:, oc], cb[oc * 128 : (oc + 1) * 128].rearrange("p -> p 1")
                )

        # --- input: DMA lands fp32 sub-tiles in a small staging pool;
        # GpSimd casts/assembles them into one contiguous bf16 image tile
        # x_full[b][ck] = [128, N]. Attention (transposes/out2 stationaries)
        # and the Winograd-1D input transform both read x_full, so no padded
        # image is kept at all (horizontal edges are AP-handled, vertical
        # edges become partial-range matmuls). ---
        NS = N // XS  # tokens per DMA sub-tile
        # x is stored de-interleaved by column parity: x_eo[.., 0, :] holds
        # even image columns, x_eo[.., 1, :] odd ones (bf16). The gram/logits
        # accumulation is token-permutation-invariant, so attention consumes
        # E/O tiles directly; only out2's DRAM writes need a strided target.
        # The payoff: every Winograd V-transform op reads contiguously (DVE
        # strided bf16 reads measured 4x slower than contiguous).
        x_eo = [
            [
                xp_pool.tile(
                    [128, 2, N // 2], BF16, tag=f"x{b}{ck}", name=f"x_eo{b}_{ck}"
                )
                for ck in range(CK)
            ]
            for b in range(BL)
        ]
        # All x assembly (fp32 stage -> bf16 x_full) runs on ACT, which is
        # otherwise idle during the gram phases. Batch 0 is emitted up front;
        # batch 1 is drip-fed through g_gram(0)'s loop so the copies never
        # sit in front of attention-critical ACT work.
        # 1024-token stage tiles halve the Sync descriptor-gen load of the
        # x stream. Even columns (consumed first by the gram E-windows) are
        # extracted on ACT; odd columns have a later deadline and go to the
        # otherwise-idle GpSimd.
        SS = 4  # stage sub-tiles per (batch, chunk)
        NSS = N // SS
        def emit_asm(b, s, ck):
            stg = stg_pool.tile([128, NSS], F32, tag="stg")
            nc.sync.dma_start(
                stg[:],
                x[b, ck * 128 : (ck + 1) * 128, s * NSS : (s + 1) * NSS],
            )
            h = NSS // 2
            nc.scalar.copy(
                x_eo[b][ck][:, 0, s * h : (s + 1) * h], stg[:, 0:NSS:2]
            )
            if b == 0:
                nc.vector.tensor_copy(
                    x_eo[b][ck][:, 1, s * h : (s + 1) * h], stg[:, 1:NSS:2]
                )
            else:
                nc.scalar.copy(
                    x_eo[b][ck][:, 1, s * h : (s + 1) * h], stg[:, 1:NSS:2]
                )

        for s in range(SS):
            for ck in range(CK):
                emit_asm(0, s, ck)
            if s == 0:
                # ident lands between the first and second stage chunks --
                # after the x stream's lead descriptors, well before the
                # first transpose needs it
                nc.sync.dma_start(ident_sb[:], ident[:])
        b1_asm = [
            (lambda s=s, ck=ck: emit_asm(1, s, ck))
            for s in range(SS)
            for ck in range(CK)
        ]
        for kc in range(CK):
            nc.sync.dma_start(
                wqk_sb[:, kc, :], wqk_t[kc * 128 : (kc + 1) * 128, :]
            )
            nc.sync.dma_start(
                wv_sb[:, kc, :], wv[kc * 128 : (kc + 1) * 128, :]
            )
            nc.sync.dma_start(
                pw_sb[:, kc, :], pw_t[kc * 128 : (kc + 1) * 128, :]
            )
        for kc in range(CK):
            nc.sync.dma_start(
                uw_sb[kc][:],
                uw[:, kc * 128 : (kc + 1) * 128, :].rearrange("t p o -> p t o"),
            )

        def tok_window(b, ck, t):
            # lhsT (stationary): [128 chan, 128 tokens] bf16. t in [0,16) maps
            # to even-column tokens, [16,32) to odd ones (a fixed permutation
            # of the token axis -- transparent to gram/logits, and out2
            # compensates with a strided DRAM write).
            par, tt = divmod(t, TT // 2)
            return x_eo[b][ck][:, par, tt * 128 : (tt + 1) * 128]

        mt_sbs = {}
        r_sbs = {}
        lg_pss = {}
        a_sbs = {}

        wqkb_holder = {}

        def qk_phase(b):
            # ---- fused [Q|K] + logits ----
            if "t" not in wqkb_holder:
                wqkb = w_pool.tile([128, CK, 2 * C], BF16, tag="wqkb")
                for kc in range(CK):
                    nc.vector.tensor_copy(wqkb[:, kc, :], wqk_sb[:, kc, :].bitcast(F32))
                wqkb_holder["t"] = wqkb
            wqkb = wqkb_holder["t"]
            lg_ps = at_ps_pool.tile(
                [128, CK, C], F32, tag="atps", name=f"lg_ps{b}"
            )
            for t in range(TT):
                qk_ps = mm_ps_pool.tile([128, 2 * C], F32, tag="qkps")
                for kc in range(CK):
                    nc.tensor.matmul(
                        qk_ps[:],
                        tok_window(b, kc, t),
                        wqkb[:, kc, :],
                        start=(kc == 0),
                        stop=(kc == CK - 1 and not use_qkv_bias),
                    )
                if use_qkv_bias:
                    nc.tensor.matmul(
                        qk_ps[:], ones1[:], bqk_sb[:], start=False, stop=True
                    )
                qk_sb = qk_pool.tile([128, 2 * C], F32R, tag="qksb")
                nc.vector.tensor_copy(qk_sb[:], qk_ps[:].bitcast(F32R))

                for cc in range(CK):
                    mm = nc.tensor.matmul(
                        lg_ps[:, cc, :],
                        qk_sb[:, cc * 128 : (cc + 1) * 128],
                        qk_sb[:, C : 2 * C],
                        start=(t == 0 and cc == 0),
                        stop=(t == TT - 1),
                        skip_group_check=True,
                    )
                    # start=True clears the WHOLE bank; order sibling groups
                    # after the clearing matmul
                    if t == 0 and cc == 0:
                        lg_clear = mm
                    elif t == 0:
                        add_dep_helper(
                            mm.ins, lg_clear.ins, sync=False,
                            reason="after lg bank clear",
                        )

            lg_pss[b] = lg_ps

        # --- Gram/logits phase, split into parts so the scheduler can
        # zipper batches: transposes are emitted ahead of gram matmuls, and
        # the serial g_sb/t1_sb copy latencies of one batch are covered by
        # independent PE work of the other. ---
        g_state = {}
        GDEPTH = 2  # transpose lookahead over gram consumption

        def g_init(b):
            # NOTE: must be called only after the previous batch's g_sb copy
            # (chain1) is emitted -- at_ps has bufs=2. Transposes may already
            # have been emitted for this batch; keep their state.
            st = g_state.setdefault(b, dict(xt={}, nextT=0))
            st["g_ps"] = at_ps_pool.tile(
                [128, CK, C], F32, tag="atps", name=f"g_ps{b}"
            )
            st["g_clear"] = None

        def g_transposes(b, upto):
            st = g_state.setdefault(b, dict(xt={}, nextT=0))
            while st["nextT"] < min(upto, TT):
                t = st["nextT"]
                st["nextT"] += 1
                xt_ps = mm_ps_pool.tile([128, C], BF16, tag="qkps")
                tclear = None
                for ck in range(CK):
                    mm = nc.tensor.matmul(
                        xt_ps[:, ck * 128 : (ck + 1) * 128],
                        tok_window(b, ck, t),
                        ident_sb[:],
                        is_transpose=True,
                        start=(ck == 0),
                        stop=(ck == CK - 1),
                        skip_group_check=True,
                    )
                    if ck == 0:
                        tclear = mm
                    else:
                        add_dep_helper(
                            mm.ins, tclear.ins, sync=False,
                            reason="after xt bank clear",
                        )
                xt_sb = qk_pool.tile([128, C], BF16, tag="qksb")
                nc.vector.tensor_copy(xt_sb[:], xt_ps[:])
                st["xt"][t] = xt_sb

        def g_gram(b, feed=None):
            st = g_state[b]
            for t in range(TT):
                if feed and t % 4 == 1:
                    feed.pop(0)()
                g_transposes(b, t + 1 + GDEPTH)
                xt_sb = st["xt"].pop(t)
                for cc in range(CK):
                    mm = nc.tensor.matmul(
                        st["g_ps"][:, cc, :],
                        xt_sb[:, cc * 128 : (cc + 1) * 128],
                        xt_sb[:],
                        start=(t == 0 and cc == 0),
                        stop=(t == TT - 1),
                        skip_group_check=True,
                    )
                    if t == 0 and cc == 0:
                        st["g_clear"] = mm
                    elif t == 0:
                        add_dep_helper(
                            mm.ins, st["g_clear"].ins, sync=False,
                            reason="after g bank clear",
                        )

        def g_chain1(b):
            # g_ps -> g_sb, then T1 = G Wk^T
            st = g_state[b]
            g_ps = st["g_ps"]
            g_sb = attn_pool.tile([128, CK, C], F32R, tag="g", name=f"g_sb{b}")
            nc.vector.tensor_copy(g_sb[:, 0, :], g_ps[:, 0, :].bitcast(F32R))
            nc.vector.tensor_copy(g_sb[:, 1, :], g_ps[:, 1, :].bitcast(F32R))
            t1_ps = mm_ps_pool.tile(
                [128, CK, C], F32, tag="qkps", name=f"t1_ps{b}"
            )
            t1_clear = None
            for cpc in range(CK):
                for dc in range(CK):
                    mm = nc.tensor.matmul(
                        t1_ps[:, cpc, :],
                        g_sb[:, dc, cpc * 128 : (cpc + 1) * 128],
                        wqk_sb[:, dc, C : 2 * C],
                        start=(cpc == 0 and dc == 0),
                        stop=(dc == CK - 1),
                        skip_group_check=True,
                    )
                    if cpc == 0 and dc == 0:
                        t1_clear = mm
                    elif dc == 0:
                        add_dep_helper(
                            mm.ins, t1_clear.ins, sync=False,
                            reason="after t1 bank clear",
                        )
            st["t1_ps"] = t1_ps

        def g_chain2(b):
            # t1_ps -> t1_sb, then logits = Wq_s T1
            st = g_state[b]
            t1_ps = st.pop("t1_ps")
            t1_sb = attn_pool.tile([128, CK, C], F32R, tag="t1", name=f"t1_sb{b}")
            nc.vector.tensor_copy(t1_sb[:, 0, :], t1_ps[:, 0, :].bitcast(F32R))
            nc.vector.tensor_copy(t1_sb[:, 1, :], t1_ps[:, 1, :].bitcast(F32R))
            lg_ps = at_ps_pool.tile(
                [128, CK, C], F32, tag="atps", name=f"glg_ps{b}"
            )
            lg_clear = None
            for cc in range(CK):
                for kc in range(CK):
                    mm = nc.tensor.matmul(
                        lg_ps[:, cc, :],
                        wqk_sb[:, kc, cc * 128 : (cc + 1) * 128],
                        t1_sb[:, kc, :],
                        start=(cc == 0 and kc == 0),
                        stop=(kc == CK - 1),
                        skip_group_check=True,
                    )
                    if cc == 0 and kc == 0:
                        lg_clear = mm
                    elif kc == 0:
                        add_dep_helper(
                            mm.ins, lg_clear.ins, sync=False,
                            reason="after glg bank clear",
                        )
            lg_pss[b] = lg_ps

        def g_phase(b):
            g_init(b)
            g_gram(b)
            g_chain1(b)
            g_chain2(b)

        def softmax_phase(b):
            lg_ps = lg_pss[b]
            # ---- softmax rows of logits -> A  [c part, d free] ----
            a_sb = attn_pool.tile([128, CK, C], F32R, tag="a", name=f"a_sb{b}")
            for cc in range(CK):
                ex = sm_pool.tile([128, C], F32, tag="ex")
                nmx = sm_pool.tile([128, 1], F32, tag=f"nmx{cc}", name=f"nmx{b}_{cc}")
                nc.vector.reduce_max(
                    nmx[:], lg_ps[:, cc, :], axis=mybir.AxisListType.X, negate=True
                )
                sm = sm_pool.tile([128, 1], F32, tag=f"sm{cc}", name=f"sm{b}_{cc}")
                nc.scalar.activation(
                    ex[:],
                    lg_ps[:, cc, :],
                    mybir.ActivationFunctionType.Exp,
                    bias=nmx[:],
                    scale=1.0,
                    accum_out=sm[:],
                )
                rs = sm_pool.tile([128, 1], F32, tag=f"rs{cc}", name=f"rs{b}_{cc}")
                nc.vector.reciprocal(rs[:], sm[:])
                nc.vector.tensor_scalar_mul(a_sb[:, cc, :], ex[:], rs[:])

            a_sbs[b] = a_sb

        u_sbs = {}

        def u_part(b):
            a_sb = a_sbs[b]
            # ---- U = A^T P^T : U[d, e] = sum_c A[c, d] pw_t[c, e] ----
            u_sb = attn_pool.tile([128, CK, C], F32R, tag="u", name=f"u_sb{b}")
            u_ps = mm_ps_pool.tile([128, CK, C], F32, tag="qkps", name=f"u_ps{b}")
            for dc in range(CK):
                for cc in range(CK):
                    mm = nc.tensor.matmul(
                        u_ps[:, dc, :],
                        a_sb[:, cc, dc * 128 : (dc + 1) * 128],
                        pw_sb[:, cc, :],
                        start=(dc == 0 and cc == 0),
                        stop=(cc == CK - 1),
                        skip_group_check=True,
                    )
                    if dc == 0 and cc == 0:
                        u_clear = mm
                    elif cc == 0:
                        add_dep_helper(
                            mm.ins, u_clear.ins, sync=False,
                            reason="after u bank clear",
                        )
            nc.vector.tensor_copy(u_sb[:, 0, :], u_ps[:, 0, :].bitcast(F32R))
            nc.scalar.copy(u_sb[:, 1, :], u_ps[:, 1, :].bitcast(F32R))
            u_sbs[b] = u_sb

        def mt_part(b):
            u_sb = u_sbs[b]
            # ---- M^T = Wv^T U : M^T[c', e] = sum_d wv[d, c'] U[d, e] ----
            mt_sb = attn_pool.tile([128, CK, C], BF16, tag="mt", name=f"mt_sb{b}")
            mt_ps = mm_ps_pool.tile(
                [128, CK, C], F32, tag="qkps", name=f"mt_ps{b}"
            )
            for cpc in range(CK):
                for dc in range(CK):
                    mm = nc.tensor.matmul(
                        mt_ps[:, cpc, :],
                        wv_sb[:, dc, cpc * 128 : (cpc + 1) * 128],
                        u_sb[:, dc, :],
                        start=(cpc == 0 and dc == 0),
                        stop=(dc == CK - 1),
                        skip_group_check=True,
                    )
                    if cpc == 0 and dc == 0:
                        mt_clear = mm
                    elif dc == 0:
                        add_dep_helper(
                            mm.ins, mt_clear.ins, sync=False,
                            reason="after mt bank clear",
                        )
            nc.vector.tensor_copy(mt_sb[:, 0, :], mt_ps[:, 0, :])
            nc.scalar.copy(mt_sb[:, 1, :], mt_ps[:, 1, :])
            mt_sbs[b] = mt_sb

        def out2_part(b, v_feed=None):
            u_sb = u_sbs[b]
            mt_sb = mt_sbs[b]
            # ---- r^T = bv^T U + pb ----
            use_r = use_v_bias or use_proj_bias
            r_sb = None
            if use_r:
                r_ps = mm_ps_pool.tile([1, C], F32, tag="qkps")
                started = False
                if use_v_bias:
                    for dc in range(CK):
                        nc.tensor.matmul(
                            r_ps[:],
                            bv_sb[:, dc],
                            u_sb[:, dc, :],
                            start=(dc == 0),
                            stop=(dc == CK - 1 and not use_proj_bias),
                        )
                    started = True
                if use_proj_bias:
                    nc.tensor.matmul(
                        r_ps[:],
                        ones1[0:1, 0:1],
                        pb_sb[:],
                        start=not started,
                        stop=True,
                    )
                r_sb = attn_pool.tile([1, C], F32R, tag="r", name=f"r_sb{b}")
                nc.vector.tensor_copy(r_sb[:], r_ps[:].bitcast(F32R))
            r_sbs[b] = r_sb

            # ---- out2[n, e] = sum_c' X[c', n] M^T[c', e] (+ 1 r^T) ----
            # PSUM drain alternates ACT/DVE so neither queue backs up and the
            # qkps pool recycles fast enough to never stall the next U chain.
            # Two token tiles share one DMA (halves Sync descriptor-gen load).
            o_sb = None
            for t in range(TT):
                o_ps = mm_ps_pool.tile([128, C], F32, tag="qkps")
                for kc in range(CK):
                    nc.tensor.matmul(
                        o_ps[:],
                        tok_window(b, kc, t),
                        mt_sb[:, kc, :],
                        start=(kc == 0),
                        stop=(kc == CK - 1 and not use_r),
                    )
                if use_r:
                    nc.tensor.matmul(
                        o_ps[:], ones1[:], r_sb[:], start=False, stop=True
                    )
                if t % 2 == 0:
                    o_sb = o2_pool.tile([128, 2, C], BF16, tag="o2sb")
                    nc.scalar.copy(o_sb[:, 0, :], o_ps[:])
                else:
                    nc.vector.tensor_copy(o_sb[:, 1, :], o_ps[:])
                    par, tt = divmod(t - 1, TT // 2)
                    nc.sync.dma_start(
                        attn_dram[b]
                        .rearrange("(j two) c -> j two c", two=2)[
                            tt * 128 : (tt + 2) * 128, par, :
                        ]
                        .rearrange("(a m) c -> m a c", a=2),
                        o_sb[:],
                    )
                if v_feed and t % 4 == 3:
                    v_feed.pop(0)()

        def rest_phase(b):
            u_part(b)
            mt_part(b)
            out2_part(b)

        # ---- Winograd-1D F(2,3) conv: horizontal transform only.
        # y[:, i, 2j]   = M0 + M1 + M2
        # y[:, i, 2j+1] = M1 - M2 - M3
        # M_v[k, i, j] = sum_{dy, c} U[v, dy][c, k] * V_v[c, i + dy - 1, j]
        # V planes are row-local (per-row strided column combos of x), the
        # vertical taps accumulate in PSUM via row-shifted APs of the same
        # V planes, and vertical image edges become partial-range matmuls.
        v_tiles = [
            [
                [
                    xp_pool.tile(
                        [128, H, 32], BF16, tag=f"v{b}{ck}{v}",
                        name=f"v{b}_{ck}_{v}",
                    )
                    for v in range(4)
                ]
                for ck in range(CK)
            ]
            for b in range(BL)
        ]

        def wino_v_ops(b, eng=None):
            # V planes from de-interleaved E/O -- every op reads contiguously.
            #   V0[j] = O[j-1] - O[j]   (V0[0] = -O[0])
            #   V1    = E + O
            #   V2    = O - E
            #   V3[j] = E[j] - E[j+1]   (V3[31] = E[31])
            if eng is None:
                eng = nc.vector
            ops = []
            for ck in range(CK):
                ev = x_eo[b][ck][:, 0, :].rearrange("p (h j) -> p h j", h=H)
                ov = x_eo[b][ck][:, 1, :].rearrange("p (h j) -> p h j", h=H)
                v0, v1, v2, v3 = (v_tiles[b][ck][v][:] for v in range(4))
                ops.append(lambda v1=v1, ev=ev, ov=ov: eng.tensor_add(
                    v1[:, :, :], ev[:, :, :], ov[:, :, :]))
                ops.append(lambda v2=v2, ev=ev, ov=ov: eng.tensor_sub(
                    v2[:, :, :], ov[:, :, :], ev[:, :, :]))

                def v0_emit(v0=v0, ov=ov):
                    eng.tensor_sub(
                        v0[:, :, 1:32], ov[:, :, 0:31], ov[:, :, 1:32]
                    )
                    eng.tensor_scalar_mul(v0[:, :, 0:1], ov[:, :, 0:1], -1.0)
                ops.append(v0_emit)

                def v3_emit(v3=v3, ev=ev):
                    eng.tensor_sub(
                        v3[:, :, 0:31], ev[:, :, 0:31], ev[:, :, 1:32]
                    )
                    eng.tensor_copy(v3[:, :, 31:32], ev[:, :, 31:32])
                ops.append(v3_emit)
            return ops

        wino_ct = {}

        def wino_groups(b, tiles, v_feed=None):
            # 16-row groups: each M_v plane fills one full PSUM bank
            # ([128, 512], 512-free matmuls stream ~12% faster than 256-free).
            # M0/M1 come from the qkps pool (freed early in the drain),
            # M2/M3 from the cv pool (freed last) -- with 6 banks total this
            # keeps two groups in flight.
            attn_chw = attn_dram[b].rearrange("(p q) c -> p q c", p=C)
            for oc, hh in tiles:
                psM = []
                for v in range(4):
                    pool, tag = (
                        (mm_ps_pool, "qkps") if v < 2 else (cv_ps_pool, "cvps")
                    )
                    psM.append(
                        pool.tile([128, 512], F32, tag=tag, name=f"w{b}{oc}{hh}{v}")
                    )
                # prefetch the attention readback while the matmuls stream
                ar = ard_pool.tile([128, 4, C], BF16, tag="ar")
                nc.sync.dma_start(
                    ar[:],
                    attn_chw[oc * 128 : (oc + 1) * 128, hh * 4 : hh * 4 + 4, :],
                )
                for v in range(4):
                    clear = None
                    for dy in (1, 0, 2):
                        r0 = hh * 16 + dy - 1
                        lo, hi = max(r0, 0), min(r0 + 16, H)
                        for ic in range(CK):
                            mm = nc.tensor.matmul(
                                psM[v][:, (lo - r0) * 32 : (hi - r0) * 32],
                                uw_sb[ic][
                                    :, v * 3 + dy, oc * 128 : (oc + 1) * 128
                                ],
                                v_tiles[b][ic][v][:, lo:hi, :],
                                start=(clear is None),
                                stop=(dy == 2 and ic == CK - 1),
                                skip_group_check=True,
                            )
                            if clear is None:
                                clear = mm
                            else:
                                add_dep_helper(
                                    mm.ins, clear.ins, sync=False,
                                    reason="after wino bank clear",
                                )
                co = cvo_pool.tile([128, 16 * W], F32, tag="co")
                cov = co[:].rearrange("p (r w) -> p r w", r=16)
                arv = ar[:].rearrange("p a (r w) -> p (a r) w", r=4)
                m = [
                    psM[v][:].rearrange("p (r j) -> p r j", r=16)
                    for v in range(4)
                ]
                co_e = cov[:, :, 0:64:2]
                co_o = cov[:, :, 1:64:2]
                nc.vector.tensor_add(co_e, arv[:, :, 0:64:2], m[0])
                nc.vector.tensor_add(co_e, co_e, m[1])
                nc.vector.tensor_add(co_e, co_e, m[2])
                nc.vector.tensor_add(co_o, arv[:, :, 1:64:2], m[1])
                nc.vector.tensor_sub(co_o, co_o, m[2])
                nc.vector.tensor_sub(co_o, co_o, m[3])
                if use_conv_bias:
                    nc.vector.tensor_scalar_add(co[:], co[:], cb_sb[:, oc])
                nc.sync.dma_start(
                    out[
                        b,
                        oc * 128 : (oc + 1) * 128,
                        hh * 1024 : (hh + 1) * 1024,
                    ],
                    co[:],
                )
                if v_feed:
                    v_feed.pop(0)()

        def conv_phase(b, tiles=None, gp_add=False):
            all_tiles = [(oc, hh) for oc in range(CK) for hh in range(4)]
            wino_groups(b, tiles if tiles is not None else all_tiles)

        front = qk_phase if use_qkv_bias else g_phase
        if use_qkv_bias or BL == 1:
            for op in b1_asm:
                op()
            b1_asm = []
            front(0)
            softmax_phase(0)
            if BL > 1:
                front(1)
            rest_phase(0)
            if BL > 1:
                softmax_phase(1)
                rest_phase(1)
            for b in range(BL):
                for op in wino_v_ops(b):
                    op()
                conv_phase(b)
        else:
            # Optimized schedule. The PE queue is in-order, so every serial
            # copy latency (g_sb/t1_sb/u/mt PSUM->SBUF, softmax) is covered
            # by emitting independent PE work from the other batch or from
            # wino-conv(0) right behind it. conv(0) groups may only be
            # emitted after out2(0) (their attn readback depends on those
            # writes). V-transform DVE ops are drip-fed into DVE-slack spots.
            all_tiles0 = [(oc, hh) for oc in range(CK) for hh in range(4)]
            v0_ops = wino_v_ops(0)
            g_init(0)
            g_gram(0, feed=b1_asm)
            g_transposes(1, 3)   # covers g_sb(0) copy latency
            g_chain1(0)
            g_transposes(1, 6)   # covers t1_sb(0) copy latency
            g_chain2(0)
            softmax_phase(0)
            g_init(1)            # after chain1(0): at_ps has bufs=2
            g_gram(1)
            u_part(0)
            g_chain1(1)
            mt_part(0)
            g_chain2(1)
            out2_part(0, v_feed=v0_ops)
            softmax_phase(1)
            wino_groups(0, all_tiles0[0:1])
            u_part(1)
            wino_groups(0, all_tiles0[1:2])
            mt_part(1)
            wino_groups(0, all_tiles0[2:3])
            out2_part(1)
            wino_groups(0, all_tiles0[3:])
            # V(1) after the b0 groups: keeps DVE free for their PSUM drains
            # (V(1) is only needed by the b1 groups, ~30us later)
            for op in wino_v_ops(1):
                op()
            wino_groups(1, all_tiles0)

    nc.compile()
    return nc


def _prep_inputs(x, qkv_w, qkv_b, proj_w, proj_b, conv_w, conv_b):
    f = np.float32
    x = np.ascontiguousarray(x, dtype=f).reshape(B, C, N)
    qkv_w = np.asarray(qkv_w, dtype=f)
    qkv_b = np.asarray(qkv_b, dtype=f)
    proj_w = np.asarray(proj_w, dtype=f)
    proj_b = np.asarray(proj_b, dtype=f)
    conv_w = np.asarray(conv_w, dtype=f)
    conv_b = np.asarray(conv_b, dtype=f)

    # [Wq*s | Wk] transposed: [256 in, 512 out] (scale folded into Q side)
    wqk_t = np.ascontiguousarray(
        np.concatenate([(qkv_w[:C] * SCALE).T, qkv_w[C : 2 * C].T], axis=1)
    )
    wv = np.ascontiguousarray(qkv_w[2 * C :])
    pw_t = np.ascontiguousarray(proj_w.T)
    # Winograd-1D F(2,3) weight transform (horizontal): G @ w_taps
    g1 = np.array(
        [[1, 0, 0], [0.5, 0.5, 0.5], [0.5, -0.5, 0.5], [0, 0, 1]], dtype=f
    )
    uw = np.einsum("vx,kcyx->vyck", g1, conv_w).reshape(12, C, C)
    uw = np.ascontiguousarray(uw).astype(ml_dtypes.bfloat16)

    bqk = np.ascontiguousarray(
        np.concatenate([qkv_b[:C] * SCALE, qkv_b[C : 2 * C]])
    )
    bv = np.ascontiguousarray(qkv_b[2 * C :])

    flags = dict(
        use_qkv_bias=bool(np.any(bqk)),
        use_v_bias=bool(np.any(bv)),
        use_proj_bias=bool(np.any(proj_b)),
        use_conv_bias=bool(np.any(conv_b)),
    )
    shared = {
        "wqk_t": wqk_t,
        "wv": wv,
        "pw_t": pw_t,
        "uw": uw,
        "ident": np.eye(128, dtype=f).astype(ml_dtypes.bfloat16),
    }
    if flags["use_qkv_bias"]:
        shared["bqk"] = bqk
    if flags["use_v_bias"]:
        shared["bv"] = bv
    if flags["use_proj_bias"]:
        shared["pb"] = proj_b
    if flags["use_conv_bias"]:
        shared["cb"] = conv_b

    in_maps = []
    for core in range(N_CORES):
        m = dict(shared)
        m["x"] = np.ascontiguousarray(x[core * BL : (core + 1) * BL])
        in_maps.append(m)
    return in_maps, flags


def run(inputs, trace=False):
    in_maps, flags = _prep_inputs(**inputs)
    nc = build_program(**flags)
    res = run_bass_kernel_spmd(nc, in_maps, list(range(N_CORES)), trace=trace)
    out = np.concatenate(
        [res.results[i]["out"].reshape(BL, C, H, W) for i in range(N_CORES)], axis=0
    )
    return out, res


def kernel(**inputs):
    out, _ = run(inputs, trace=False)
    return out

